# revision 24
# baseline (speedup 1.0000x reference)
"""Trainium2 Bass kernel for BinaryTimedPSP (causal boxcar window sum + clip).

psp[t] = clip(sum_{k=max(0,t-D+1)}^{t} x[k], 0, 1) along time axis of a
[T=2048, B=16, N=2048] f32 spike tensor, D = duration (100).

Strategy (v2): pure data-parallel over 8 NeuronCores; each core owns a
[T, 4096] slab of the flattened B*N axis.
  - input is cast to fp8e4 on the host (0/1 values are exact) -> 4x less
    HBM read traffic than f32
  - the whole slab lives in SBUF as one [128, 16, 4096] tile; time chunk i
    is written by its own DMA, and the window sum of chunk i is ONE
    DoubleRow fp8 matmul with K=256: ktile0 = chunk i-1 (band block A1),
    ktile1 = chunk i (band block A0). Chunk 0 uses a weight tile whose
    second k-tile is zero. 0.5 cycles/row = 4x the f32r matmul rate.
  - PSUM eviction (the clip) is split across two engines: DVE does
    tensor_scalar_min(.,1) on the low half, Act does activation Sign on
    the high half (sums are >= 0 so sign(s) == min(s,1) exactly).
  - outputs are exactly {0,1} so fp8e4 stores are bit-exact; the host
    gather restores f32 losslessly.
No cross-core communication; the gather is a host-side concatenate.
"""

import numpy as np

T_FULL, B_FULL, N_FULL = 2048, 16, 2048
NCORES = 8
P = 128
COLS = B_FULL * N_FULL          # 32768
FREE = COLS // NCORES           # 4096 columns per core
NCHUNK = T_FULL // P            # 16 time chunks
EV = 2048                       # eviction tile: 4 PSUM banks of f32
FTILE = 512                     # one PSUM bank of f32 (matmul out width)

_CACHE: dict = {}


def _band_weights(d: int) -> np.ndarray:
    """[2, 128, 2, 128] fp8 lhsT weights: [which, c(part), ktile, r(free)].

    which=0 (main, chunks i>=1): ktile0 = A1^T (applies to chunk i-1),
                                 ktile1 = A0^T (chunk i)
    which=1 (first, chunk 0):    ktile0 = A0^T (chunk 0), ktile1 = 0
    A0[r,c] = 1 iff 0 <= r-c < d ;  A1[r,c] = 1 iff 0 <= r+128-c < d
    """
    import ml_dtypes

    r = np.arange(P)[None, :]
    c = np.arange(P)[:, None]
    a0t = ((r - c >= 0) & (r - c < d)).astype(np.float32)        # [c, r]
    a1t = ((r + P - c >= 0) & (r + P - c < d)).astype(np.float32)
    w = np.zeros((2, P, 2, P), np.float32)
    w[0, :, 0, :] = a1t
    w[0, :, 1, :] = a0t
    w[1, :, 0, :] = a0t
    return w.astype(ml_dtypes.float8_e4m3)


def _build(d: int):
    import concourse.bacc as bacc
    import concourse.mybir as mybir
    from concourse.tile import TileContext

    f32 = mybir.dt.float32
    f8 = mybir.dt.float8e4
    DR = mybir.MatmulPerfMode.DoubleRow
    Sign = mybir.ActivationFunctionType.Sign

    nc = bacc.Bacc(None)
    x = nc.dram_tensor("x", [T_FULL, FREE], f8, kind="ExternalInput")
    w = nc.dram_tensor("w", [2 * P, 2 * P], f8, kind="ExternalInput")
    y = nc.dram_tensor("y", [T_FULL, FREE], f8, kind="ExternalOutput")
    xr = x.rearrange("(n p) f -> n p f", p=P)
    yr = y.rearrange("(n p) f -> n p f", p=P)
    wr = w.rearrange("(m p) (k r) -> p m k r", p=P, k=2)

    QV = 1024                   # psum tile: 2 banks of f32
    NQ = FREE // QV             # 4 psum tiles per chunk
    # the tail of the pipeline is DVE-stream-paced (2.37us/chunk vs Act's
    # 2.11); handing one mid-stream chunk's third tile to Act equalizes them
    ACT_HEAVY = (10,)

    with nc.allow_low_precision("values are exactly 0/1; fp8e4 is lossless"), TileContext(nc) as tc:
        with (
            tc.tile_pool(name="wpool", bufs=1) as wpool,
            tc.tile_pool(name="xpool", bufs=1) as xpool,
            tc.tile_pool(name="opool", bufs=12) as opool,
            tc.tile_pool(name="ppool", bufs=4, space="PSUM") as ppool,
        ):
            # Per-DMA trigger cost (~0.6us on the sequencer) dominates small
            # transfers, so most chunks load as 1MB chunk-pair DMAs. The
            # exception is the head: a small load covering just psum-tile
            # q0's columns of chunks 0+1 gates the very first matmuls, then
            # the (tiny) weights, then the rest of chunks 0+1. The slab is
            # persistent so loads have no hazards.
            # both weight tiles ride in one DMA (one tile, sliced) to save a
            # trigger slot in the latency-critical head
            wboth = wpool.tile([P, 2, 2, P], f8, tag="w")
            wm = wboth[:, 0, :, :]
            wf = wboth[:, 1, :, :]
            slab = xpool.tile([P, NCHUNK, FREE], f8, tag="slab")
            xp = x.rearrange("(n p) f -> p n f", p=P)
            nc.sync.dma_start(out=slab[:, 0:2, 0:QV], in_=xp[:, 0:2, 0:QV])
            nc.sync.dma_start(out=wboth, in_=wr)
            nc.sync.dma_start(out=slab[:, 0:2, QV:], in_=xp[:, 0:2, QV:])
            for i in range(2, NCHUNK, 2):
                nc.sync.dma_start(out=slab[:, i : i + 2, :], in_=xp[:, i : i + 2, :])

            # PE warmup: zero matmuls on a memset tile while the first loads
            # are still in flight, so the PE pstate ramps before real work
            warm = wpool.tile([P, 2, P], f8, tag="warm")
            nc.vector.memset(warm, 0.0)
            wps = ppool.tile([P, QV], f32, tag="ps")
            for _ in range(6):
                nc.tensor.matmul(
                    wps[:, 0:P], warm, warm, start=True, stop=True, perf_mode=DR,
                    skip_group_check=True,
                )

            for i in range(NCHUNK):
                # rhs k-tile pair: (chunk i-1, chunk i); chunk 0 pairs with
                # chunk 1 but its weight k-tile1 is zero so the value is
                # ignored (only adds a dep on load 1, which is early anyway)
                lo = i - 1 if i > 0 else 0
                wt = wm if i > 0 else wf
                drain = i == NCHUNK - 1
                ot = opool.tile([P, FREE], f8, tag="o")
                for q in range(NQ):
                    ps = ppool.tile([P, QV], f32, tag="ps")
                    for f in range(QV // FTILE):
                        cs = q * QV + f * FTILE
                        nc.tensor.matmul(
                            ps[:, f * FTILE : (f + 1) * FTILE],
                            wt,
                            slab[:, lo : lo + 2, cs : cs + FTILE],
                            start=True,
                            stop=True,
                            perf_mode=DR,
                        )
                    cs = q * QV
                    # Act owns the low columns (its act-table load finishes
                    # during DMA warmup) -- except chunk 0, where DVE (no
                    # table load) takes the low half to start immediately
                    on_dve = q >= (3 if i in ACT_HEAVY else 2)
                    if i == 0:
                        on_dve = q < 2
                    if on_dve:
                        # DVE evicts with the clip fused into the copy
                        nc.vector.tensor_scalar_min(
                            out=ot[:, cs : cs + QV], in0=ps, scalar1=1.0
                        )
                    else:
                        # Act evicts via Sign: sums are >= 0 integers so
                        # sign(s) == min(s, 1) exactly
                        nc.scalar.activation(
                            out=ot[:, cs : cs + QV], in_=ps, func=Sign
                        )
                    if drain and q % 2 == 1:
                        # drain the tail in halves: each half-store waits on
                        # just one engine's evict pair
                        nc.sync.dma_start(
                            out=yr[i][:, cs - QV : cs + QV],
                            in_=ot[:, cs - QV : cs + QV],
                        )
                if not drain:
                    nc.sync.dma_start(out=yr[i], in_=ot)
    nc.finalize()
    return nc


def _get_built(d: int):
    if d not in _CACHE:
        _CACHE[d] = _build(d)
    return _CACHE[d]


def kernel(input_spikes, duration, _trace=False):
    import ml_dtypes
    from concourse.bass_utils import run_bass_kernel_spmd

    d = int(duration)
    # the fused DoubleRow band matmul covers windows up to 129 rows back
    assert 1 <= d <= P + 1, d
    x = np.asarray(input_spikes)
    assert x.shape == (T_FULL, B_FULL, N_FULL), x.shape

    nc = _get_built(d)
    W = _band_weights(d).reshape(2 * P, 2 * P)

    # exact host-side cast: spikes are {0.0, 1.0}; 1.0 in fp8e4m3 is 0x38
    f8 = ml_dtypes.float8_e4m3
    xb = (np.asarray(x, dtype=np.float32).reshape(T_FULL, COLS) != 0).astype(
        np.uint8
    ) * np.uint8(0x38)
    in_maps = [
        {
            "x": np.ascontiguousarray(xb[:, c * FREE : (c + 1) * FREE]).view(f8),
            "w": W,
        }
        for c in range(NCORES)
    ]
    res = run_bass_kernel_spmd(
        nc, in_maps, core_ids=list(range(NCORES)), trace=_trace
    )
    out = np.concatenate([r["y"] for r in res.results], axis=1)
    out = out.astype(np.float32).reshape(T_FULL, B_FULL, N_FULL)
    if _trace:
        return out, res
    return out


# revision 27
# speedup vs baseline: 1.0083x; 1.0083x over previous
"""Trainium2 Bass kernel for BinaryTimedPSP (causal boxcar window sum + clip).

psp[t] = clip(sum_{k=max(0,t-D+1)}^{t} x[k], 0, 1) along time axis of a
[T=2048, B=16, N=2048] f32 spike tensor, D = duration (100).

Strategy (v2): pure data-parallel over 8 NeuronCores; each core owns a
[T, 4096] slab of the flattened B*N axis.
  - input is cast to fp8e4 on the host (0/1 values are exact) -> 4x less
    HBM read traffic than f32
  - the whole slab lives in SBUF as one [128, 16, 4096] tile; time chunk i
    is written by its own DMA, and the window sum of chunk i is ONE
    DoubleRow fp8 matmul with K=256: ktile0 = chunk i-1 (band block A1),
    ktile1 = chunk i (band block A0). Chunk 0 uses a weight tile whose
    second k-tile is zero. 0.5 cycles/row = 4x the f32r matmul rate.
  - PSUM eviction (the clip) is split across two engines: DVE does
    tensor_scalar_min(.,1) on the low half, Act does activation Sign on
    the high half (sums are >= 0 so sign(s) == min(s,1) exactly).
  - outputs are exactly {0,1} so fp8e4 stores are bit-exact; the host
    gather restores f32 losslessly.
No cross-core communication; the gather is a host-side concatenate.
"""

import numpy as np

T_FULL, B_FULL, N_FULL = 2048, 16, 2048
NCORES = 8
P = 128
COLS = B_FULL * N_FULL          # 32768
FREE = COLS // NCORES           # 4096 columns per core
NCHUNK = T_FULL // P            # 16 time chunks
EV = 2048                       # eviction tile: 4 PSUM banks of f32
FTILE = 512                     # one PSUM bank of f32 (matmul out width)

_CACHE: dict = {}


def _band_weights(d: int) -> np.ndarray:
    """[2, 128, 2, 128] fp8 lhsT weights: [which, c(part), ktile, r(free)].

    which=0 (main, chunks i>=1): ktile0 = A1^T (applies to chunk i-1),
                                 ktile1 = A0^T (chunk i)
    which=1 (first, chunk 0):    ktile0 = A0^T (chunk 0), ktile1 = 0
    A0[r,c] = 1 iff 0 <= r-c < d ;  A1[r,c] = 1 iff 0 <= r+128-c < d
    """
    import ml_dtypes

    r = np.arange(P)[None, :]
    c = np.arange(P)[:, None]
    a0t = ((r - c >= 0) & (r - c < d)).astype(np.float32)        # [c, r]
    a1t = ((r + P - c >= 0) & (r + P - c < d)).astype(np.float32)
    w = np.zeros((2, P, 2, P), np.float32)
    w[0, :, 0, :] = a1t
    w[0, :, 1, :] = a0t
    w[1, :, 0, :] = a0t
    return w.astype(ml_dtypes.float8_e4m3)


def _build(d: int):
    import concourse.bacc as bacc
    import concourse.mybir as mybir
    from concourse.tile import TileContext

    f32 = mybir.dt.float32
    f8 = mybir.dt.float8e4
    DR = mybir.MatmulPerfMode.DoubleRow
    Sign = mybir.ActivationFunctionType.Sign

    nc = bacc.Bacc(None)
    x = nc.dram_tensor("x", [T_FULL, FREE], f8, kind="ExternalInput")
    w = nc.dram_tensor("w", [2 * P, 2 * P], f8, kind="ExternalInput")
    y = nc.dram_tensor("y", [T_FULL, FREE], f8, kind="ExternalOutput")
    xr = x.rearrange("(n p) f -> n p f", p=P)
    yr = y.rearrange("(n p) f -> n p f", p=P)
    wr = w.rearrange("(m p) (k r) -> p m k r", p=P, k=2)

    QV = 1024                   # psum tile: 2 banks of f32
    NQ = FREE // QV             # 4 psum tiles per chunk
    # the steady state is DVE-stream-paced (2.37us/chunk vs Act's 2.11) but
    # handing any chunk's third tile to Act serializes Act for 3.2us there,
    # which measured worse; keep the symmetric 2+2 split
    ACT_HEAVY = ()

    with nc.allow_low_precision("values are exactly 0/1; fp8e4 is lossless"), TileContext(nc) as tc:
        with (
            tc.tile_pool(name="wpool", bufs=1) as wpool,
            tc.tile_pool(name="xpool", bufs=1) as xpool,
            tc.tile_pool(name="opool", bufs=12) as opool,
            tc.tile_pool(name="ppool", bufs=4, space="PSUM") as ppool,
        ):
            # Per-DMA trigger cost (~0.6us on the sequencer) dominates small
            # transfers, so most chunks load as 1MB chunk-pair DMAs. The
            # exception is the head: a small load covering just psum-tile
            # q0's columns of chunks 0+1 gates the very first matmuls, then
            # the (tiny) weights, then the rest of chunks 0+1. The slab is
            # persistent so loads have no hazards.
            # both weight tiles ride in one DMA (one tile, sliced) to save a
            # trigger slot in the latency-critical head
            wboth = wpool.tile([P, 2, 2, P], f8, tag="w")
            wm = wboth[:, 0, :, :]
            wf = wboth[:, 1, :, :]
            slab = xpool.tile([P, NCHUNK, FREE], f8, tag="slab")
            xp = x.rearrange("(n p) f -> p n f", p=P)
            nc.sync.dma_start(out=slab[:, 0:2, 0:QV], in_=xp[:, 0:2, 0:QV])
            nc.sync.dma_start(out=wboth, in_=wr)
            nc.sync.dma_start(out=slab[:, 0:2, QV:], in_=xp[:, 0:2, QV:])
            for i in range(2, NCHUNK, 2):
                nc.sync.dma_start(out=slab[:, i : i + 2, :], in_=xp[:, i : i + 2, :])

            # PE warmup: zero matmuls on a memset tile while the first loads
            # are still in flight, so the PE pstate ramps before real work
            warm = wpool.tile([P, 2, P], f8, tag="warm")
            nc.vector.memset(warm, 0.0)
            wps = ppool.tile([P, QV], f32, tag="ps")
            for _ in range(6):
                nc.tensor.matmul(
                    wps[:, 0:P], warm, warm, start=True, stop=True, perf_mode=DR,
                    skip_group_check=True,
                )

            for i in range(NCHUNK):
                # rhs k-tile pair: (chunk i-1, chunk i); chunk 0 pairs with
                # chunk 1 but its weight k-tile1 is zero so the value is
                # ignored (only adds a dep on load 1, which is early anyway)
                lo = i - 1 if i > 0 else 0
                wt = wm if i > 0 else wf
                drain = i == NCHUNK - 1
                ot = opool.tile([P, FREE], f8, tag="o")
                for q in range(NQ):
                    ps = ppool.tile([P, QV], f32, tag="ps")
                    for f in range(QV // FTILE):
                        cs = q * QV + f * FTILE
                        nc.tensor.matmul(
                            ps[:, f * FTILE : (f + 1) * FTILE],
                            wt,
                            slab[:, lo : lo + 2, cs : cs + FTILE],
                            start=True,
                            stop=True,
                            perf_mode=DR,
                        )
                    cs = q * QV
                    # Act owns the low columns (its act-table load finishes
                    # during DMA warmup) -- except chunk 0, where DVE (no
                    # table load) takes the low half to start immediately
                    on_dve = q >= (3 if i in ACT_HEAVY else 2)
                    if i == 0:
                        on_dve = q < 2
                    if on_dve:
                        # DVE evicts with the clip fused into the copy
                        nc.vector.tensor_scalar_min(
                            out=ot[:, cs : cs + QV], in0=ps, scalar1=1.0
                        )
                    else:
                        # Act evicts via Sign: sums are >= 0 integers so
                        # sign(s) == min(s, 1) exactly
                        nc.scalar.activation(
                            out=ot[:, cs : cs + QV], in_=ps, func=Sign
                        )
                    if drain and q % 2 == 1:
                        # drain the tail in halves: each half-store waits on
                        # just one engine's evict pair
                        nc.sync.dma_start(
                            out=yr[i][:, cs - QV : cs + QV],
                            in_=ot[:, cs - QV : cs + QV],
                        )
                if not drain:
                    nc.sync.dma_start(out=yr[i], in_=ot)
    nc.finalize()
    return nc


def _get_built(d: int):
    if d not in _CACHE:
        _CACHE[d] = _build(d)
    return _CACHE[d]


def kernel(input_spikes, duration, _trace=False):
    import ml_dtypes
    from concourse.bass_utils import run_bass_kernel_spmd

    d = int(duration)
    # the fused DoubleRow band matmul covers windows up to 129 rows back
    assert 1 <= d <= P + 1, d
    x = np.asarray(input_spikes)
    assert x.shape == (T_FULL, B_FULL, N_FULL), x.shape

    nc = _get_built(d)
    W = _band_weights(d).reshape(2 * P, 2 * P)

    # exact host-side cast: spikes are {0.0, 1.0}; 1.0 in fp8e4m3 is 0x38
    f8 = ml_dtypes.float8_e4m3
    xb = (np.asarray(x, dtype=np.float32).reshape(T_FULL, COLS) != 0).astype(
        np.uint8
    ) * np.uint8(0x38)
    in_maps = [
        {
            "x": np.ascontiguousarray(xb[:, c * FREE : (c + 1) * FREE]).view(f8),
            "w": W,
        }
        for c in range(NCORES)
    ]
    res = run_bass_kernel_spmd(
        nc, in_maps, core_ids=list(range(NCORES)), trace=_trace
    )
    out = np.concatenate([r["y"] for r in res.results], axis=1)
    out = out.astype(np.float32).reshape(T_FULL, B_FULL, N_FULL)
    if _trace:
        return out, res
    return out
